# revision 2
# baseline (speedup 1.0000x reference)
"""Multi-head attention (B=2, S=2048, D=1024, H=16) on 8 Trainium2 NeuronCores.

Sharding: core c = (batch b = c//4) x (head-group g = c%4, 4 heads each).
Each core computes its 4 heads' attention plus the partial output
projection over its 256 W_o rows; the host sums the group partials.

All matmuls run in fp16 (end-to-end max rel err ~1e-3 vs the fp32
reference). PSUM accumulation is fp32.

v2 layout: every matmul runs in the PE's (128,128) tiling mode so the
kc loop never pays a tiling-mode-switch drain. The per-head K tiles are
stored zero-PADDED to 128 contraction rows (head 2g at partitions 0-63
with zeros below, head 2g+1 at partitions 64-127 with zeros above, in
disjoint column halves); the padded rows multiply garbage Q rows by
zero, so only the lhsT (K) side needs the memset. Scores stream time is
column-bound, so the padding costs nothing.

Emission plan: the PE is the bottleneck engine (~164us of matmul column
streams vs ~143us of ScalarE exp), so stage-2 (QKV) and stage-4 (W_o)
work is chopped into self-contained ~0.4-0.9us pieces and dropped into
the attention kc loop so the PE never idles. Strands run qt-major so
output-projection pieces unlock as early as possible; the last quarter
of the projection is computed per-hp (host adds the extra partial) to
shrink the no-exp tail.
"""

import sys

for _p in ("/opt/trn_rl_repo", "/root/.axon_site/_ro/trn_rl_repo"):
    if _p not in sys.path:
        sys.path.insert(0, _p)

import numpy as np

import concourse.mybir as mybir
import concourse.tile as tile
from concourse import bacc
from concourse.bass_utils import run_bass_kernel_spmd

F32 = mybir.dt.float32
F16 = mybir.dt.float16

B, S, D = 2, 2048, 1024
H, DK = 16, 64
HPC = 4          # heads per core
NCORES = 8
DC = 8           # number of 128-row chunks of D (contraction tiles)
SC = 4           # S chunks of 512 for the projections
QT_W = 1024      # q-tile width in stage 3
KC = S // 128    # 16 k-chunks
V_W = DK + 1     # 65: V columns per head incl. fused ones column

_CACHED_NC = None


def _build_nc():
    nc = bacc.Bacc("TRN2", target_bir_lowering=False, debug=False)

    xs = nc.dram_tensor("xs", [SC, 128, DC * 512], F16, kind="ExternalInput")
    wq = nc.dram_tensor("wq", [128, DC * 2 * 128], F16, kind="ExternalInput")
    wk = nc.dram_tensor("wk", [128, DC * 2 * 128], F16, kind="ExternalInput")
    wv = nc.dram_tensor("wv", [128, DC * HPC * DK], F16, kind="ExternalInput")
    wo = nc.dram_tensor("wo", [2, 128, D], F16, kind="ExternalInput")
    out = nc.dram_tensor("out", [S, D], F16, kind="ExternalOutput")
    # hp=1 partial of the qt=1 rows (host adds it onto out[1024:])
    out2 = nc.dram_tensor("out2", [QT_W, D], F16, kind="ExternalOutput")

    with tile.TileContext(nc) as tc:
        with (
            tc.tile_pool(name="persist", bufs=1) as pp,
            tc.tile_pool(name="ps_mm", bufs=2, space="PSUM") as ps_mm,
            tc.tile_pool(name="ps_acc", bufs=4, space="PSUM") as ps_acc,
            tc.tile_pool(name="exp_pool", bufs=8) as ep,
            tc.tile_pool(name="out_pool", bufs=2) as op_,
            tc.tile_pool(name="nrm_pool", bufs=5) as np_,
        ):
            # ---- ScalarE act-table preload: dummy exp before anything else
            warm_in = pp.tile([128, 1], F32, tag="warm_i")
            warm_out = pp.tile([128, 1], F16, tag="warm_o")
            nc.gpsimd.memset(warm_in[:], 0.0)
            nc.scalar.activation(
                warm_out[:], warm_in[:], mybir.ActivationFunctionType.Exp
            )

            # ---- input DMAs, ordered so the first strand's deps land first
            wk_sb = pp.tile([128, DC * 256], F16, tag="wk")
            nc.sync.dma_start(wk_sb[:], wk.ap())
            x_sb = [
                pp.tile([128, DC * 512], F16, tag=f"x{i}", name=f"x_sb{i}")
                for i in range(SC)
            ]
            nc.sync.dma_start(x_sb[0][:], xs.ap()[0])
            wq_sb = pp.tile([128, DC * 256], F16, tag="wq")
            nc.sync.dma_start(wq_sb[:], wq.ap())
            nc.sync.dma_start(x_sb[1][:], xs.ap()[1])
            wv_sb = pp.tile([128, DC * 256], F16, tag="wv")
            nc.sync.dma_start(wv_sb[:], wv.ap())
            nc.sync.dma_start(x_sb[2][:], xs.ap()[2])
            nc.sync.dma_start(x_sb[3][:], xs.ap()[3])
            wo_sb = [
                pp.tile([128, D], F16, tag=f"wo{i}", name=f"wo_sb{i}")
                for i in range(2)
            ]
            for i in range(2):
                nc.sync.dma_start(wo_sb[i][:], wo.ap()[i])

            # ---- per-head-pair Q/K tiles, 2 heads in disjoint column halves
            # head 2g at partitions 0-63 (cols 0:S), head 2g+1 at partitions
            # 64-127 (cols S:2S). kt needs zeros in the complement rows (it is
            # the matmul lhsT); qt complement rows are never read as nonzero
            # weights so they can stay garbage.
            qt_sb = [
                pp.tile([128, 2 * S], F16, tag=f"qt{i}", name=f"qt_sb{i}")
                for i in range(2)
            ]
            kt_sb = [
                pp.tile([128, 2 * S], F16, tag=f"kt{i}", name=f"kt_sb{i}")
                for i in range(2)
            ]
            vp_sb = pp.tile([128, KC * HPC * V_W], F16, tag="vp")
            ot_sb = [
                pp.tile([128, S], F16, tag=f"ot{i}", name=f"ot_sb{i}")
                for i in range(2)
            ]

            # zero scratch -> kt pad halves (bounce via f32: memset is f32-only)
            zero_sb = pp.tile([128, 1024], F32, tag="zero")
            nc.gpsimd.memset(zero_sb[:], 0.0)
            for hp in range(2):
                for half in range(2):
                    nc.vector.tensor_copy(
                        kt_sb[hp][64:128, half * 1024 : (half + 1) * 1024],
                        zero_sb[64:128, :],
                    )
                for half in range(2):
                    nc.vector.tensor_copy(
                        kt_sb[hp][0:64, S + half * 1024 : S + (half + 1) * 1024],
                        zero_sb[0:64, :],
                    )

            # ones columns of V'
            ones_sb = pp.tile([128, KC * HPC], F32, tag="ones")
            nc.gpsimd.memset(ones_sb[:], 1.0)
            ones_ap = vp_sb[:].rearrange("p (c g) -> p c g", g=V_W)[:, :, DK : DK + 1]
            nc.vector.tensor_copy(ones_ap, ones_sb[:].unsqueeze(-1))

            # ---- stage-2 / stage-4 work units. Each piece is self-contained
            # (allocates and releases its PSUM slot within the piece) so a
            # piece can sit anywhere in the PE queue without deadlocking the
            # ps_mm rotation.
            def qk_piece(w_sb, t_sb, hp, sc, colhalf):
                """Half of a Q/K projection unit: 256 of the 512 sc columns."""
                ps = ps_mm.tile([128, 512], F32, tag="mm", name="ps_qk")
                c0 = colhalf * 256
                for d in range(DC):
                    nc.tensor.matmul(
                        ps[:, 0:256],
                        w_sb[:, d * 256 + hp * 128 : d * 256 + hp * 128 + 128],
                        x_sb[sc][:, d * 512 + c0 : d * 512 + c0 + 256],
                        start=(d == 0),
                        stop=(d == DC - 1),
                    )
                # rows 0-63 = head 2g -> cols [sc*512+c0 ..], rows 64-127 =
                # head 2g+1 -> same cols offset by S
                nc.vector.tensor_copy(
                    t_sb[hp][0:64, sc * 512 + c0 : sc * 512 + c0 + 256],
                    ps[0:64, 0:256],
                )
                nc.vector.tensor_copy(
                    t_sb[hp][64:128, S + sc * 512 + c0 : S + sc * 512 + c0 + 256],
                    ps[64:128, 0:256],
                )

            def v_unit(kc):
                sc, i = divmod(kc, 4)
                ps = ps_mm.tile([128, 512], F32, tag="mm", name="ps_v")
                for d in range(DC):
                    nc.tensor.matmul(
                        ps[:, 0 : HPC * DK],
                        x_sb[sc][:, d * 512 + i * 128 : d * 512 + i * 128 + 128],
                        wv_sb[:, d * 256 : (d + 1) * 256],
                        start=(d == 0),
                        stop=(d == DC - 1),
                    )
                dst = vp_sb[:, kc * V_W * HPC : (kc + 1) * V_W * HPC]
                dst = dst.rearrange("p (g c) -> p g c", c=V_W)[:, :, 0:DK]
                src = ps[:, 0 : HPC * DK].rearrange("p (g c) -> p g c", c=DK)
                nc.vector.tensor_copy(dst, src)

            # stage 4: full unit (hp-summed on device) for qt0 rows; per-hp
            # half for qt1 rows (host adds the hp=1 partial from out2).
            def s4_full_piece(q16, dc2, o_sb):
                ps = ps_mm.tile([128, 512], F32, tag="mm", name="ps_s4")
                for hp in range(2):
                    nc.tensor.matmul(
                        ps[:],
                        ot_sb[hp][:, q16 * 128 : (q16 + 1) * 128],
                        wo_sb[hp][:, dc2 * 512 : (dc2 + 1) * 512],
                        start=(hp == 0),
                        stop=(hp == 1),
                    )
                nc.vector.tensor_copy(o_sb[:, dc2 * 512 : (dc2 + 1) * 512], ps[:])
                if dc2 == 1:
                    nc.sync.dma_start(
                        out.ap()[q16 * 128 : (q16 + 1) * 128, :], o_sb[:]
                    )

            def s4_half_piece(q16, hp, dc2, o_sb):
                ps = ps_mm.tile([128, 512], F32, tag="mm", name="ps_s4h")
                nc.tensor.matmul(
                    ps[:],
                    ot_sb[hp][:, q16 * 128 : (q16 + 1) * 128],
                    wo_sb[hp][:, dc2 * 512 : (dc2 + 1) * 512],
                    start=True,
                    stop=True,
                )
                nc.vector.tensor_copy(o_sb[:, dc2 * 512 : (dc2 + 1) * 512], ps[:])
                if dc2 == 1:
                    if hp == 0:
                        nc.sync.dma_start(
                            out.ap()[q16 * 128 : (q16 + 1) * 128, :], o_sb[:]
                        )
                    else:
                        r0 = q16 * 128 - QT_W
                        nc.sync.dma_start(out2.ap()[r0 : r0 + 128, :], o_sb[:])

            # s4 emitters keeping o_sb across the two dc2 pieces
            s4_state = {}

            def s4_piece(kind, q16, hp=None):
                key = (kind, q16, hp)
                if key not in s4_state:
                    s4_state[key] = op_.tile([128, D], F16, tag="o", name="o_sb")
                    dc2 = 0
                else:
                    dc2 = 1
                o_sb = s4_state[key]
                if kind == "full":
                    s4_full_piece(q16, dc2, o_sb)
                else:
                    s4_half_piece(q16, hp, dc2, o_sb)

            # ---- prologue stage-2 work: just enough for strand 0 kc0
            for colhalf in range(2):
                qk_piece(wk_sb, kt_sb, 0, 0, colhalf)
            for colhalf in range(2):
                qk_piece(wq_sb, qt_sb, 0, 0, colhalf)
            for colhalf in range(2):
                qk_piece(wq_sb, qt_sb, 0, 1, colhalf)

            # ---- filler schedule: (strand_idx, kc) -> list of thunks
            fillers = {}

            def F(si, kc, fn):
                fillers.setdefault((si, kc), []).append(fn)

            # strand 0: remaining K(hp0), all K(hp1), Q(hp1) for qt0, all V
            s0_qk = (
                [(wk_sb, kt_sb, 0, sc) for sc in (1, 2, 3)]
                + [(wk_sb, kt_sb, 1, sc) for sc in (0, 1)]
                + [(wq_sb, qt_sb, 1, sc) for sc in (0, 1)]
            )
            for u, (w, t, hp, sc) in enumerate(s0_qk):
                for colhalf in range(2):
                    F(0, 2 * u + colhalf, lambda w=w, t=t, hp=hp, sc=sc,
                      c=colhalf: qk_piece(w, t, hp, sc, c))
            F(0, 0, lambda: v_unit(0))
            F(0, 1, lambda: v_unit(1))
            for k in range(2, 16):
                F(0, k, lambda k=k: v_unit(k))
            # strand 1: K(hp1) sc2-3, Q(hp0) sc2-3, Q(hp1) sc2-3
            s1_qk = (
                [(wk_sb, kt_sb, 1, sc) for sc in (2, 3)]
                + [(wq_sb, qt_sb, 0, sc) for sc in (2, 3)]
                + [(wq_sb, qt_sb, 1, sc) for sc in (2, 3)]
            )
            for u, (w, t, hp, sc) in enumerate(s1_qk):
                for colhalf in range(2):
                    F(1, 2 * u + colhalf, lambda w=w, t=t, hp=hp, sc=sc,
                      c=colhalf: qk_piece(w, t, hp, sc, c))
            # strand 2: projection of qt0 rows (norms of strands 0,1 done)
            for u in range(16):
                F(2, u, lambda q16=u // 2: s4_piece("full", q16))
            # strand 3: hp0 half of qt1 rows (strand 2 norm done)
            for u in range(16):
                F(3, u, lambda q16=8 + u // 2: s4_piece("half", q16, 0))

            # ---- attention strands, qt-major: (qt, hp)
            strands = [(0, 0), (0, 1), (1, 0), (1, 1)]
            PV_LAG = 2

            for si, (qt, hp) in enumerate(strands):
                accs = {}
                for hsel in range(2):
                    for j in range(2):
                        accs[hsel, j] = ps_acc.tile(
                            [128, 512], F32, tag="acc", name=f"acc{hsel}{j}"
                        )

                def pv(kc, es, accs=accs, hp=hp):
                    for hsel in range(2):
                        h = hp * 2 + hsel
                        lhsT = vp_sb[
                            :, (kc * HPC + h) * V_W : (kc * HPC + h) * V_W + V_W
                        ]
                        for j in range(2):
                            nc.tensor.matmul(
                                accs[hsel, j][0:V_W, :],
                                lhsT,
                                es[hsel][:, j * 512 : (j + 1) * 512],
                                start=(kc == 0),
                                stop=(kc == KC - 1),
                            )

                pending = []
                for kc in range(KC):
                    es = []
                    for hsel in range(2):
                        sc_ps = ps_mm.tile([128, QT_W], F32, tag="mm")
                        for j in range(2):
                            nc.tensor.matmul(
                                sc_ps[:, j * 512 : (j + 1) * 512],
                                kt_sb[hp][
                                    :, hsel * S + kc * 128 : hsel * S + (kc + 1) * 128
                                ],
                                qt_sb[hp][
                                    :,
                                    hsel * S
                                    + qt * QT_W
                                    + j * 512 : hsel * S
                                    + qt * QT_W
                                    + (j + 1) * 512,
                                ],
                                start=True,
                                stop=True,
                            )
                        e_sb = ep.tile([128, QT_W], F16, tag="e")
                        nc.scalar.activation(
                            e_sb[:], sc_ps[:], mybir.ActivationFunctionType.Exp
                        )
                        es.append(e_sb)
                    pending.append((kc, es))
                    if len(pending) > PV_LAG:
                        pv(*pending.pop(0))
                    for u in fillers.get((si, kc), ()):
                        u()
                for item in pending:
                    pv(*item)

                # normalize, software-pipelined across the 4 accumulators
                quads = [(hsel, j) for j in range(2) for hsel in range(2)]
                dens, rs, rbs = {}, {}, {}
                for hsel, j in quads:
                    den_sb = np_.tile([1, 512], F32, tag="den", name=f"den{hsel}{j}")
                    nc.vector.tensor_copy(den_sb[:], accs[hsel, j][DK : DK + 1, :])
                    dens[hsel, j] = den_sb
                for hsel, j in quads:
                    r_sb = np_.tile([1, 512], F32, tag="r", name=f"r{hsel}{j}")
                    nc.vector.reciprocal_approx_fast(r_sb[:], dens[hsel, j][:])
                    rs[hsel, j] = r_sb
                for hsel, j in quads:
                    rb_sb = np_.tile([64, 512], F32, tag="rb", name=f"rb{hsel}{j}")
                    nc.gpsimd.partition_broadcast(rb_sb[:], rs[hsel, j][:])
                    rbs[hsel, j] = rb_sb
                for hsel, j in quads:
                    q0 = qt * QT_W + j * 512
                    nc.vector.tensor_mul(
                        ot_sb[hp][hsel * 64 : hsel * 64 + 64, q0 : q0 + 512],
                        accs[hsel, j][0:DK, :],
                        rbs[hsel, j][:],
                    )

            # ---- tail: hp1 half of qt1 projection rows
            for u in range(16):
                s4_piece("half", 8 + u // 2, 1)

    nc.compile()
    return nc


def _shard_inputs(x, W_q, W_k, W_v, W_o):
    """Build the 8 per-core input maps (fp16, C-contiguous)."""

    def pack_w(w_rows):  # [256, D] weight rows -> [128, DC*256] lhsT tiles
        wt = w_rows.T.astype(np.float16)  # [D, 256]
        return np.ascontiguousarray(
            wt.reshape(DC, 128, 256).transpose(1, 0, 2).reshape(128, DC * 256)
        )

    in_maps = []
    for c in range(NCORES):
        b, g = divmod(c, HPC)
        rows = slice(g * HPC * DK, (g + 1) * HPC * DK)
        xt = x[b].T.astype(np.float16)  # [D, S]
        xs = np.ascontiguousarray(
            xt.reshape(DC, 128, SC, 512).transpose(2, 1, 0, 3).reshape(SC, 128, DC * 512)
        )
        in_maps.append(
            {
                "xs": xs,
                "wq": pack_w(W_q[rows] * 0.125),
                "wk": pack_w(W_k[rows]),
                "wv": pack_w(W_v[rows]),
                "wo": np.ascontiguousarray(
                    W_o[:, rows].T.astype(np.float16).reshape(2, 128, D)
                ),
            }
        )
    return in_maps


def _numpy_fallback(x, attention_mask, W_q, W_k, W_v, W_o):
    """Exact reference path (only used if the mask is not all ones)."""
    out = np.empty((B, S, D), np.float32)
    for b in range(B):
        q = (x[b] @ W_q.T).reshape(S, H, DK).transpose(1, 0, 2)
        k = (x[b] @ W_k.T).reshape(S, H, DK).transpose(1, 0, 2)
        v = (x[b] @ W_v.T).reshape(S, H, DK).transpose(1, 0, 2)
        scores = np.einsum("hqd,hkd->hqk", q, k)
        scores = np.where(attention_mask[b][None, None, :] == 0, -np.inf, scores)
        scores = scores / np.sqrt(DK)
        scores -= scores.max(axis=-1, keepdims=True)
        w = np.exp(scores)
        w /= w.sum(axis=-1, keepdims=True)
        o = np.einsum("hqk,hkd->hqd", w, v).transpose(1, 0, 2).reshape(S, D)
        out[b] = o @ W_o.T
    return out


def kernel(x, attention_mask, W_q, W_k, W_v, W_o, _trace=False):
    global _CACHED_NC
    x = np.asarray(x, dtype=np.float32)
    attention_mask = np.asarray(attention_mask)
    W_q = np.asarray(W_q, dtype=np.float32)
    W_k = np.asarray(W_k, dtype=np.float32)
    W_v = np.asarray(W_v, dtype=np.float32)
    W_o = np.asarray(W_o, dtype=np.float32)

    if not np.all(attention_mask == 1):
        return _numpy_fallback(x, attention_mask, W_q, W_k, W_v, W_o)

    if _CACHED_NC is None:
        _CACHED_NC = _build_nc()
    nc = _CACHED_NC

    in_maps = _shard_inputs(x, W_q, W_k, W_v, W_o)
    res = run_bass_kernel_spmd(
        nc, in_maps, core_ids=list(range(NCORES)), trace=_trace
    )

    out = np.empty((B, S, D), np.float32)
    for b in range(B):
        acc = np.zeros((S, D), np.float64)
        for g in range(HPC):
            r = res.results[b * HPC + g]
            acc += r["out"].astype(np.float64)
            acc[QT_W:] += r["out2"].astype(np.float64)
        out[b] = acc.astype(np.float32)
    if _trace:
        kernel.last_exec_time_ns = res.exec_time_ns
    return out


# revision 12
# speedup vs baseline: 1.0396x; 1.0396x over previous
"""Multi-head attention (B=2, S=2048, D=1024, H=16) on 8 Trainium2 NeuronCores.

Sharding: core c = (batch b = c//4) x (head-group g = c%4, 4 heads each).
Each core computes its 4 heads' attention plus the partial output
projection over its 256 W_o rows; the host sums the group partials.

All matmuls run in fp16 (end-to-end max rel err ~1e-3 vs the fp32
reference). PSUM accumulation is fp32.

v2 layout: every matmul runs in the PE's (128,128) tiling mode so the
kc loop never pays a tiling-mode-switch drain. The per-head K tiles are
stored zero-PADDED to 128 contraction rows (head 2g at partitions 0-63
with zeros below, head 2g+1 at partitions 64-127 with zeros above, in
disjoint column halves); the padded rows multiply garbage Q rows by
zero, so only the lhsT (K) side needs the memset. Scores stream time is
column-bound, so the padding costs nothing.

Emission plan: the PE is the bottleneck engine (~164us of matmul column
streams vs ~143us of ScalarE exp), so stage-2 (QKV) and stage-4 (W_o)
work is chopped into self-contained ~0.4-0.9us pieces and dropped into
the attention kc loop so the PE never idles. Strands run qt-major so
output-projection pieces unlock as early as possible; the last quarter
of the projection is computed per-hp (host adds the extra partial) to
shrink the no-exp tail.
"""

import sys

for _p in ("/opt/trn_rl_repo", "/root/.axon_site/_ro/trn_rl_repo"):
    if _p not in sys.path:
        sys.path.insert(0, _p)

import numpy as np

import concourse.mybir as mybir
import concourse.tile as tile
from concourse import bacc
from concourse.bass_utils import run_bass_kernel_spmd

F32 = mybir.dt.float32
F16 = mybir.dt.float16

B, S, D = 2, 2048, 1024
H, DK = 16, 64
HPC = 4          # heads per core
NCORES = 8
DC = 8           # number of 128-row chunks of D (contraction tiles)
SC = 4           # S chunks of 512 for the projections
QT_W = 1024      # q-tile width in stage 3
KC = S // 128    # 16 k-chunks
V_W = DK + 1     # 65: V columns per head incl. fused ones column

_CACHED_NC = None


def _build_nc():
    nc = bacc.Bacc("TRN2", target_bir_lowering=False, debug=False)

    xs = nc.dram_tensor("xs", [SC, 128, DC * 512], F16, kind="ExternalInput")
    wq = nc.dram_tensor("wq", [128, DC * 2 * 128], F16, kind="ExternalInput")
    wk = nc.dram_tensor("wk", [128, DC * 2 * 128], F16, kind="ExternalInput")
    wv = nc.dram_tensor("wv", [128, DC * HPC * DK], F16, kind="ExternalInput")
    wo = nc.dram_tensor("wo", [2, 128, D], F16, kind="ExternalInput")
    out = nc.dram_tensor("out", [S, D], F16, kind="ExternalOutput")
    # hp=1 partial of the qt=0 rows (host adds it onto out[:1024])
    out2 = nc.dram_tensor("out2", [QT_W, D], F16, kind="ExternalOutput")

    with tile.TileContext(nc) as tc:
        with (
            tc.tile_pool(name="persist", bufs=1) as pp,
            tc.tile_pool(name="ps_mm", bufs=2, space="PSUM") as ps_mm,
            tc.tile_pool(name="ps_acc", bufs=4, space="PSUM") as ps_acc,
            tc.tile_pool(name="exp_pool", bufs=8) as ep,
            tc.tile_pool(name="out_pool", bufs=2) as op_,
            tc.tile_pool(name="nrm_pool", bufs=5) as np_,
        ):
            # ---- ScalarE act-table preload: dummy exp before anything else
            warm_in = pp.tile([128, 1], F32, tag="warm_i")
            warm_out = pp.tile([128, 1], F16, tag="warm_o")
            nc.gpsimd.memset(warm_in[:], 0.0)
            nc.scalar.activation(
                warm_out[:], warm_in[:], mybir.ActivationFunctionType.Exp
            )

            # ---- input DMAs, ordered so the first strand's deps land first
            wk_sb = pp.tile([128, DC * 256], F16, tag="wk")
            nc.sync.dma_start(wk_sb[:], wk.ap())
            x_sb = [
                pp.tile([128, DC * 512], F16, tag=f"x{i}", name=f"x_sb{i}")
                for i in range(SC)
            ]
            nc.sync.dma_start(x_sb[0][:], xs.ap()[0])
            wq_sb = pp.tile([128, DC * 256], F16, tag="wq")
            nc.sync.dma_start(wq_sb[:], wq.ap())
            nc.sync.dma_start(x_sb[1][:], xs.ap()[1])
            wv_sb = pp.tile([128, DC * 256], F16, tag="wv")
            nc.sync.dma_start(wv_sb[:], wv.ap())
            nc.sync.dma_start(x_sb[2][:], xs.ap()[2])
            nc.sync.dma_start(x_sb[3][:], xs.ap()[3])
            wo_sb = [
                pp.tile([128, D], F16, tag=f"wo{i}", name=f"wo_sb{i}")
                for i in range(2)
            ]
            for i in range(2):
                nc.sync.dma_start(wo_sb[i][:], wo.ap()[i])

            # ---- per-head-pair Q/K tiles, 2 heads in disjoint column halves
            # head 2g at partitions 0-63 (cols 0:S), head 2g+1 at partitions
            # 64-127 (cols S:2S). kt needs zeros in the complement rows (it is
            # the matmul lhsT); qt complement rows are never read as nonzero
            # weights so they can stay garbage.
            qt_sb = [
                pp.tile([128, 2 * S], F16, tag=f"qt{i}", name=f"qt_sb{i}")
                for i in range(2)
            ]
            kt_sb = [
                pp.tile([128, 2 * S], F16, tag=f"kt{i}", name=f"kt_sb{i}")
                for i in range(2)
            ]
            vp_sb = pp.tile([128, KC * HPC * V_W], F16, tag="vp")
            ot_sb = [
                pp.tile([128, S], F16, tag=f"ot{i}", name=f"ot_sb{i}")
                for i in range(2)
            ]

            # zero scratch -> kt/qt pad halves (bounce via f32: memset is
            # f32-only). kt pads make the padded contraction exact; qt pads
            # guard against Inf/NaN garbage turning 0*garbage into NaN.
            zero_sb = pp.tile([128, 1024], F32, tag="zero")
            nc.gpsimd.memset(zero_sb[:], 0.0)
            zf_sb = pp.tile([128, 512], F16, tag="zf")
            nc.vector.tensor_copy(zf_sb[:], zero_sb[:, 0:512])
            for t_sb in (kt_sb[0], qt_sb[0], kt_sb[1], qt_sb[1]):
                for half in range(2):
                    nc.vector.tensor_copy(
                        t_sb[64:128, half * 1024 : (half + 1) * 1024],
                        zero_sb[64:128, :],
                    )
                for half in range(2):
                    nc.vector.tensor_copy(
                        t_sb[0:64, S + half * 1024 : S + (half + 1) * 1024],
                        zero_sb[0:64, :],
                    )

            # dummy matmuls on the zero tile: keep the PE_HAM activity window
            # busy (idle >=3.4us re-throttles the PE clock to 1.2GHz)
            def pe_warm(n):
                ps = ps_mm.tile([128, 512], F32, tag="mm", name="ps_warm")
                for i in range(n):
                    nc.tensor.matmul(
                        ps[:], zf_sb[:, 0:128], zf_sb[:], start=True, stop=True
                    )

            # warm the PE during the input-DMA wait so the first real matmul
            # already runs at 2.4GHz
            pe_warm(8)
            pe_warm(8)

            # ones columns of V'
            ones_sb = pp.tile([128, KC * HPC], F32, tag="ones")
            nc.gpsimd.memset(ones_sb[:], 1.0)
            ones_ap = vp_sb[:].rearrange("p (c g) -> p c g", g=V_W)[:, :, DK : DK + 1]
            nc.vector.tensor_copy(ones_ap, ones_sb[:].unsqueeze(-1))

            # ---- stage-2 / stage-4 work units. Each unit is self-contained
            # (allocates and releases its PSUM slot within the unit) so a
            # unit can sit anywhere in the PE queue without deadlocking the
            # ps_mm rotation.
            def qk_unit(w_sb, t_sb, hp, sc):
                """Q or K projection for one head pair over 512 queries."""
                ps = ps_mm.tile([128, 512], F32, tag="mm", name="ps_qk")
                for d in range(DC):
                    nc.tensor.matmul(
                        ps[:],
                        w_sb[:, d * 256 + hp * 128 : d * 256 + hp * 128 + 128],
                        x_sb[sc][:, d * 512 : (d + 1) * 512],
                        start=(d == 0),
                        stop=(d == DC - 1),
                    )
                # rows 0-63 = head 2g -> cols [sc*512 ..], rows 64-127 =
                # head 2g+1 -> same cols offset by S
                nc.vector.tensor_copy(
                    t_sb[hp][0:64, sc * 512 : (sc + 1) * 512], ps[0:64, :]
                )
                nc.vector.tensor_copy(
                    t_sb[hp][64:128, S + sc * 512 : S + (sc + 1) * 512],
                    ps[64:128, :],
                )

            def v_unit(kc):
                sc, i = divmod(kc, 4)
                ps = ps_mm.tile([128, 512], F32, tag="mm", name="ps_v")
                for d in range(DC):
                    nc.tensor.matmul(
                        ps[:, 0 : HPC * DK],
                        x_sb[sc][:, d * 512 + i * 128 : d * 512 + i * 128 + 128],
                        wv_sb[:, d * 256 : (d + 1) * 256],
                        start=(d == 0),
                        stop=(d == DC - 1),
                    )
                dst = vp_sb[:, kc * V_W * HPC : (kc + 1) * V_W * HPC]
                dst = dst.rearrange("p (g c) -> p g c", c=V_W)[:, :, 0:DK]
                src = ps[:, 0 : HPC * DK].rearrange("p (g c) -> p g c", c=DK)
                nc.vector.tensor_copy(dst, src)

            # stage 4: full unit (hp-summed on device) for qt1 rows; per-hp
            # half for qt0 rows (host adds the hp=1 partial from out2).
            # One [128,1024] PSUM alloc + one cast + one DMA per unit keeps
            # the ps_mm rotation shallow (3 allocs per kc at most).
            def s4_full(q16):
                ps = ps_mm.tile([128, QT_W], F32, tag="mm", name="ps_s4")
                o_sb = op_.tile([128, D], F16, tag="o", name="o_sb")
                for hp in range(2):
                    for dc2 in range(2):
                        nc.tensor.matmul(
                            ps[:, dc2 * 512 : (dc2 + 1) * 512],
                            ot_sb[hp][:, q16 * 128 : (q16 + 1) * 128],
                            wo_sb[hp][:, dc2 * 512 : (dc2 + 1) * 512],
                            start=(hp == 0),
                            stop=(hp == 1),
                        )
                nc.vector.tensor_copy(o_sb[:], ps[:])
                nc.sync.dma_start(out.ap()[q16 * 128 : (q16 + 1) * 128, :], o_sb[:])

            def s4_half(q16, hp):
                ps = ps_mm.tile([128, QT_W], F32, tag="mm", name="ps_s4h")
                o_sb = op_.tile([128, D], F16, tag="o", name="o_sb")
                for dc2 in range(2):
                    nc.tensor.matmul(
                        ps[:, dc2 * 512 : (dc2 + 1) * 512],
                        ot_sb[hp][:, q16 * 128 : (q16 + 1) * 128],
                        wo_sb[hp][:, dc2 * 512 : (dc2 + 1) * 512],
                        start=True,
                        stop=True,
                    )
                nc.vector.tensor_copy(o_sb[:], ps[:])
                if hp == 0:
                    nc.sync.dma_start(
                        out.ap()[q16 * 128 : (q16 + 1) * 128, :], o_sb[:]
                    )
                else:
                    nc.sync.dma_start(out2.ap()[q16 * 128 : (q16 + 1) * 128, :], o_sb[:])

            # ---- prologue stage-2 work: just enough for strand 0 kc0
            qk_unit(wk_sb, kt_sb, 0, 0)
            qk_unit(wq_sb, qt_sb, 0, 0)
            qk_unit(wq_sb, qt_sb, 0, 1)

            # ---- filler schedule: (strand_idx, kc) -> list of thunks
            fillers = {}

            def F(si, kc, fn):
                fillers.setdefault((si, kc), []).append(fn)

            # strand 0 (qt0,hp0): rest of K(hp0) [self, kc4+], Q(hp0) sc2-3
            # [strand 1], and all 16 V chunks [PV lag 2]
            F(0, 0, lambda: qk_unit(wk_sb, kt_sb, 0, 1))
            F(0, 2, lambda: qk_unit(wk_sb, kt_sb, 0, 2))
            F(0, 4, lambda: qk_unit(wk_sb, kt_sb, 0, 3))
            F(0, 6, lambda: qk_unit(wq_sb, qt_sb, 0, 2))
            F(0, 8, lambda: qk_unit(wq_sb, qt_sb, 0, 3))
            for k in range(16):
                F(0, k, lambda k=k: v_unit(k))
            # strand 1 (qt1,hp0): K(hp1) all sc [strand 2], Q(hp1) sc2-3
            # [strand 2]
            for u, sc in enumerate(range(SC)):
                F(1, 2 * u + 1, lambda sc=sc: qk_unit(wk_sb, kt_sb, 1, sc))
            F(1, 9, lambda: qk_unit(wq_sb, qt_sb, 1, 2))
            F(1, 11, lambda: qk_unit(wq_sb, qt_sb, 1, 3))
            # strand 2 (qt1,hp1): Q(hp1) sc0-1 [strand 3], then the hp0 half
            # of the qt0 projection (strand 0 norm done)
            F(2, 1, lambda: qk_unit(wq_sb, qt_sb, 1, 0))
            F(2, 3, lambda: qk_unit(wq_sb, qt_sb, 1, 1))
            for u in range(8):
                F(2, 5 + u, lambda q16=u: s4_half(q16, 0))
            # strand 3 (qt0,hp1): full qt1 projection (strands 1,2 norms done)
            for u in range(8):
                F(3, 2 * u, lambda q16=8 + u: s4_full(q16))

            # ---- attention strands: (qt, hp) ordered so only the qt0/hp1
            # projection half is left for the tail
            strands = [(0, 0), (1, 0), (1, 1), (0, 1)]
            PV_LAG = 2

            for si, (qt, hp) in enumerate(strands):
                accs = {}
                for hsel in range(2):
                    for j in range(2):
                        accs[hsel, j] = ps_acc.tile(
                            [128, 512], F32, tag="acc", name=f"acc{hsel}{j}"
                        )

                def pv(kc, es, accs=accs, hp=hp):
                    for hsel in range(2):
                        h = hp * 2 + hsel
                        lhsT = vp_sb[
                            :, (kc * HPC + h) * V_W : (kc * HPC + h) * V_W + V_W
                        ]
                        for j in range(2):
                            nc.tensor.matmul(
                                accs[hsel, j][0:V_W, :],
                                lhsT,
                                es[hsel][:, j * 512 : (j + 1) * 512],
                                start=(kc == 0),
                                stop=(kc == KC - 1),
                            )

                pending = []
                for kc in range(KC):
                    es = []
                    for hsel in range(2):
                        sc_ps = ps_mm.tile([128, QT_W], F32, tag="mm")
                        for j in range(2):
                            nc.tensor.matmul(
                                sc_ps[:, j * 512 : (j + 1) * 512],
                                kt_sb[hp][
                                    :, hsel * S + kc * 128 : hsel * S + (kc + 1) * 128
                                ],
                                qt_sb[hp][
                                    :,
                                    hsel * S
                                    + qt * QT_W
                                    + j * 512 : hsel * S
                                    + qt * QT_W
                                    + (j + 1) * 512,
                                ],
                                start=True,
                                stop=True,
                            )
                        e_sb = ep.tile([128, QT_W], F16, tag="e")
                        nc.scalar.activation(
                            e_sb[:], sc_ps[:], mybir.ActivationFunctionType.Exp
                        )
                        es.append(e_sb)
                    pending.append((kc, es))
                    if len(pending) > PV_LAG:
                        pv(*pending.pop(0))
                    for u in fillers.get((si, kc), ()):
                        u()
                for item in pending:
                    pv(*item)

                # keep the PE_HAM window busy across the normalize latency
                # (the tail after strand 3 has no real PE work to chase)
                pe_warm(10 if si == 3 else 2)
                # normalize, software-pipelined across the 4 accumulators
                quads = [(hsel, j) for j in range(2) for hsel in range(2)]
                dens, rs, rbs = {}, {}, {}
                for hsel, j in quads:
                    den_sb = np_.tile([1, 512], F32, tag="den", name=f"den{hsel}{j}")
                    nc.vector.tensor_copy(den_sb[:], accs[hsel, j][DK : DK + 1, :])
                    dens[hsel, j] = den_sb
                for hsel, j in quads:
                    r_sb = np_.tile([1, 512], F32, tag="r", name=f"r{hsel}{j}")
                    nc.vector.reciprocal_approx_fast(r_sb[:], dens[hsel, j][:])
                    rs[hsel, j] = r_sb
                for hsel, j in quads:
                    rb_sb = np_.tile([64, 512], F32, tag="rb", name=f"rb{hsel}{j}")
                    nc.gpsimd.partition_broadcast(rb_sb[:], rs[hsel, j][:])
                    rbs[hsel, j] = rb_sb
                for hsel, j in quads:
                    q0 = qt * QT_W + j * 512
                    nc.vector.tensor_mul(
                        ot_sb[hp][hsel * 64 : hsel * 64 + 64, q0 : q0 + 512],
                        accs[hsel, j][0:DK, :],
                        rbs[hsel, j][:],
                    )

            # ---- tail: hp1 half of qt0 projection rows
            for q16 in range(8):
                s4_half(q16, 1)

    nc.compile()
    return nc


def _shard_inputs(x, W_q, W_k, W_v, W_o):
    """Build the 8 per-core input maps (fp16, C-contiguous)."""

    def pack_w(w_rows):  # [256, D] weight rows -> [128, DC*256] lhsT tiles
        wt = w_rows.T.astype(np.float16)  # [D, 256]
        return np.ascontiguousarray(
            wt.reshape(DC, 128, 256).transpose(1, 0, 2).reshape(128, DC * 256)
        )

    in_maps = []
    for c in range(NCORES):
        b, g = divmod(c, HPC)
        rows = slice(g * HPC * DK, (g + 1) * HPC * DK)
        xt = x[b].T.astype(np.float16)  # [D, S]
        xs = np.ascontiguousarray(
            xt.reshape(DC, 128, SC, 512).transpose(2, 1, 0, 3).reshape(SC, 128, DC * 512)
        )
        in_maps.append(
            {
                "xs": xs,
                "wq": pack_w(W_q[rows] * 0.125),
                "wk": pack_w(W_k[rows]),
                "wv": pack_w(W_v[rows]),
                "wo": np.ascontiguousarray(
                    W_o[:, rows].T.astype(np.float16).reshape(2, 128, D)
                ),
            }
        )
    return in_maps


def _numpy_fallback(x, attention_mask, W_q, W_k, W_v, W_o):
    """Exact reference path (only used if the mask is not all ones)."""
    out = np.empty((B, S, D), np.float32)
    for b in range(B):
        q = (x[b] @ W_q.T).reshape(S, H, DK).transpose(1, 0, 2)
        k = (x[b] @ W_k.T).reshape(S, H, DK).transpose(1, 0, 2)
        v = (x[b] @ W_v.T).reshape(S, H, DK).transpose(1, 0, 2)
        scores = np.einsum("hqd,hkd->hqk", q, k)
        scores = np.where(attention_mask[b][None, None, :] == 0, -np.inf, scores)
        scores = scores / np.sqrt(DK)
        scores -= scores.max(axis=-1, keepdims=True)
        w = np.exp(scores)
        w /= w.sum(axis=-1, keepdims=True)
        o = np.einsum("hqk,hkd->hqd", w, v).transpose(1, 0, 2).reshape(S, D)
        out[b] = o @ W_o.T
    return out


def kernel(x, attention_mask, W_q, W_k, W_v, W_o, _trace=False):
    global _CACHED_NC
    x = np.asarray(x, dtype=np.float32)
    attention_mask = np.asarray(attention_mask)
    W_q = np.asarray(W_q, dtype=np.float32)
    W_k = np.asarray(W_k, dtype=np.float32)
    W_v = np.asarray(W_v, dtype=np.float32)
    W_o = np.asarray(W_o, dtype=np.float32)

    if not np.all(attention_mask == 1):
        return _numpy_fallback(x, attention_mask, W_q, W_k, W_v, W_o)

    if _CACHED_NC is None:
        _CACHED_NC = _build_nc()
    nc = _CACHED_NC

    in_maps = _shard_inputs(x, W_q, W_k, W_v, W_o)
    res = run_bass_kernel_spmd(
        nc, in_maps, core_ids=list(range(NCORES)), trace=_trace
    )

    out = np.empty((B, S, D), np.float32)
    for b in range(B):
        acc = np.zeros((S, D), np.float64)
        for g in range(HPC):
            r = res.results[b * HPC + g]
            acc += r["out"].astype(np.float64)
            acc[:QT_W] += r["out2"].astype(np.float64)
        out[b] = acc.astype(np.float32)
    if _trace:
        kernel.last_exec_time_ns = res.exec_time_ns
    return out


# revision 17
# speedup vs baseline: 1.0587x; 1.0184x over previous
"""Multi-head attention (B=2, S=2048, D=1024, H=16) on 8 Trainium2 NeuronCores.

Sharding: core c = (batch b = c//4) x (head-group g = c%4, 4 heads each).
Each core computes its 4 heads' attention plus the partial output
projection over its 256 W_o rows; the host sums the group partials.

All matmuls run in fp16 (end-to-end max rel err ~1e-3 vs the fp32
reference). PSUM accumulation is fp32.

v2 layout: every matmul runs in the PE's (128,128) tiling mode so the
kc loop never pays a tiling-mode-switch drain. The per-head K tiles are
stored zero-PADDED to 128 contraction rows (head 2g at partitions 0-63
with zeros below, head 2g+1 at partitions 64-127 with zeros above, in
disjoint column halves); the padded rows multiply garbage Q rows by
zero, so only the lhsT (K) side needs the memset. Scores stream time is
column-bound, so the padding costs nothing.

Emission plan: the PE is the bottleneck engine (~164us of matmul column
streams vs ~143us of ScalarE exp), so stage-2 (QKV) and stage-4 (W_o)
work is chopped into self-contained ~0.4-0.9us pieces and dropped into
the attention kc loop so the PE never idles. Strands run qt-major so
output-projection pieces unlock as early as possible; the last quarter
of the projection is computed per-hp (host adds the extra partial) to
shrink the no-exp tail.
"""

import sys

for _p in ("/opt/trn_rl_repo", "/root/.axon_site/_ro/trn_rl_repo"):
    if _p not in sys.path:
        sys.path.insert(0, _p)

import numpy as np

import concourse.mybir as mybir
import concourse.tile as tile
from concourse import bacc
from concourse.bass_utils import run_bass_kernel_spmd

F32 = mybir.dt.float32
F16 = mybir.dt.float16

B, S, D = 2, 2048, 1024
H, DK = 16, 64
HPC = 4          # heads per core
NCORES = 8
DC = 8           # number of 128-row chunks of D (contraction tiles)
SC = 4           # S chunks of 512 for the projections
QT_W = 1024      # q-tile width in stage 3
KC = S // 128    # 16 k-chunks
V_W = DK + 1     # 65: V columns per head incl. fused ones column

_CACHED_NC = None


def _build_nc():
    nc = bacc.Bacc("TRN2", target_bir_lowering=False, debug=False)

    xs = nc.dram_tensor("xs", [SC, 128, DC * 512], F16, kind="ExternalInput")
    wq = nc.dram_tensor("wq", [128, DC * 2 * 128], F16, kind="ExternalInput")
    wk = nc.dram_tensor("wk", [128, DC * 2 * 128], F16, kind="ExternalInput")
    wv = nc.dram_tensor("wv", [128, DC * HPC * DK], F16, kind="ExternalInput")
    wo = nc.dram_tensor("wo", [2, 128, D], F16, kind="ExternalInput")
    out = nc.dram_tensor("out", [S, D], F16, kind="ExternalOutput")
    # hp=1 partial of the qt=0 rows (host adds it onto out[:1024])
    out2 = nc.dram_tensor("out2", [QT_W, D], F16, kind="ExternalOutput")

    with tile.TileContext(nc) as tc:
        with (
            tc.tile_pool(name="persist", bufs=1) as pp,
            tc.tile_pool(name="ps_mm", bufs=2, space="PSUM") as ps_mm,
            tc.tile_pool(name="ps_acc", bufs=4, space="PSUM") as ps_acc,
            tc.tile_pool(name="exp_pool", bufs=8) as ep,
            tc.tile_pool(name="out_pool", bufs=2) as op_,
            tc.tile_pool(name="nrm_pool", bufs=5) as np_,
        ):
            # ---- ScalarE act-table preload: dummy exp before anything else
            warm_in = pp.tile([128, 1], F32, tag="warm_i")
            warm_out = pp.tile([128, 1], F16, tag="warm_o")
            nc.gpsimd.memset(warm_in[:], 0.0)
            nc.scalar.activation(
                warm_out[:], warm_in[:], mybir.ActivationFunctionType.Exp
            )

            # ---- input DMAs, ordered so the first strand's deps land first
            wk_sb = pp.tile([128, DC * 256], F16, tag="wk")
            nc.sync.dma_start(wk_sb[:], wk.ap())
            x_sb = [
                pp.tile([128, DC * 512], F16, tag=f"x{i}", name=f"x_sb{i}")
                for i in range(SC)
            ]
            nc.sync.dma_start(x_sb[0][:], xs.ap()[0])
            wq_sb = pp.tile([128, DC * 256], F16, tag="wq")
            nc.sync.dma_start(wq_sb[:], wq.ap())
            nc.sync.dma_start(x_sb[1][:], xs.ap()[1])
            wv_sb = pp.tile([128, DC * 256], F16, tag="wv")
            nc.sync.dma_start(wv_sb[:], wv.ap())
            nc.sync.dma_start(x_sb[2][:], xs.ap()[2])
            nc.sync.dma_start(x_sb[3][:], xs.ap()[3])
            wo_sb = [
                pp.tile([128, D], F16, tag=f"wo{i}", name=f"wo_sb{i}")
                for i in range(2)
            ]
            for i in range(2):
                nc.sync.dma_start(wo_sb[i][:], wo.ap()[i])

            # ---- per-head-pair Q/K tiles, 2 heads in disjoint column halves
            # head 2g at partitions 0-63 (cols 0:S), head 2g+1 at partitions
            # 64-127 (cols S:2S). kt needs zeros in the complement rows (it is
            # the matmul lhsT); qt complement rows are never read as nonzero
            # weights so they can stay garbage.
            qt_sb = [
                pp.tile([128, 2 * S], F16, tag=f"qt{i}", name=f"qt_sb{i}")
                for i in range(2)
            ]
            kt_sb = [
                pp.tile([128, 2 * S], F16, tag=f"kt{i}", name=f"kt_sb{i}")
                for i in range(2)
            ]
            vp_sb = pp.tile([128, KC * HPC * V_W], F16, tag="vp")
            ot_sb = [
                pp.tile([128, S], F16, tag=f"ot{i}", name=f"ot_sb{i}")
                for i in range(2)
            ]

            # dummy matmuls on kt garbage (result discarded): keep the PE_HAM
            # activity window busy (idle >=3.4us re-throttles the PE clock to
            # 1.2GHz). No data deps, so they issue from t~0.
            def pe_warm(n):
                ps = ps_mm.tile([128, 512], F32, tag="mm", name="ps_warm")
                for i in range(n):
                    nc.tensor.matmul(
                        ps[:],
                        kt_sb[0][:, 0:128],
                        kt_sb[0][:, 0:512],
                        start=True,
                        stop=True,
                    )

            # warm the PE during the input-DMA wait so the first real matmul
            # already runs at 2.4GHz
            pe_warm(5)
            pe_warm(5)

            # zero scratch -> kt/qt pad halves (bounce via f32: memset is
            # f32-only). kt pads make the padded contraction exact; qt pads
            # guard against Inf/NaN garbage turning 0*garbage into NaN.
            # hp0 tiles are needed by strand 0: their zeros go on ScalarE
            # (idle until the first exp) + DVE; hp1 tiles are zeroed by
            # strand-0 fillers.
            zero_sb = pp.tile([128, 1024], F32, tag="zero")
            nc.gpsimd.memset(zero_sb[:], 0.0)

            def pad_zero(t_sb, eng):
                for half in range(2):
                    eng(
                        t_sb[64:128, half * 1024 : (half + 1) * 1024],
                        zero_sb[64:128, :],
                    )
                for half in range(2):
                    eng(
                        t_sb[0:64, S + half * 1024 : S + (half + 1) * 1024],
                        zero_sb[0:64, :],
                    )

            pad_zero(kt_sb[0], nc.scalar.copy)
            pad_zero(qt_sb[0], nc.vector.tensor_copy)

            # ones columns of V'
            ones_sb = pp.tile([128, KC * HPC], F32, tag="ones")
            nc.gpsimd.memset(ones_sb[:], 1.0)
            ones_ap = vp_sb[:].rearrange("p (c g) -> p c g", g=V_W)[:, :, DK : DK + 1]
            nc.vector.tensor_copy(ones_ap, ones_sb[:].unsqueeze(-1))

            # ---- stage-2 / stage-4 work units. Each unit is self-contained
            # (allocates and releases its PSUM slot within the unit) so a
            # unit can sit anywhere in the PE queue without deadlocking the
            # ps_mm rotation.
            def qk_unit(w_sb, t_sb, hp, sc):
                """Q or K projection for one head pair over 512 queries."""
                ps = ps_mm.tile([128, 512], F32, tag="mm", name="ps_qk")
                for d in range(DC):
                    nc.tensor.matmul(
                        ps[:],
                        w_sb[:, d * 256 + hp * 128 : d * 256 + hp * 128 + 128],
                        x_sb[sc][:, d * 512 : (d + 1) * 512],
                        start=(d == 0),
                        stop=(d == DC - 1),
                    )
                # rows 0-63 = head 2g -> cols [sc*512 ..], rows 64-127 =
                # head 2g+1 -> same cols offset by S
                nc.vector.tensor_copy(
                    t_sb[hp][0:64, sc * 512 : (sc + 1) * 512], ps[0:64, :]
                )
                nc.vector.tensor_copy(
                    t_sb[hp][64:128, S + sc * 512 : S + (sc + 1) * 512],
                    ps[64:128, :],
                )

            def v_unit(kc):
                sc, i = divmod(kc, 4)
                ps = ps_mm.tile([128, 512], F32, tag="mm", name="ps_v")
                for d in range(DC):
                    nc.tensor.matmul(
                        ps[:, 0 : HPC * DK],
                        x_sb[sc][:, d * 512 + i * 128 : d * 512 + i * 128 + 128],
                        wv_sb[:, d * 256 : (d + 1) * 256],
                        start=(d == 0),
                        stop=(d == DC - 1),
                    )
                dst = vp_sb[:, kc * V_W * HPC : (kc + 1) * V_W * HPC]
                dst = dst.rearrange("p (g c) -> p g c", c=V_W)[:, :, 0:DK]
                src = ps[:, 0 : HPC * DK].rearrange("p (g c) -> p g c", c=DK)
                nc.vector.tensor_copy(dst, src)

            # stage 4: full unit (hp-summed on device) for qt1 rows; per-hp
            # half for qt0 rows (host adds the hp=1 partial from out2).
            # One [128,1024] PSUM alloc + one cast + one DMA per unit keeps
            # the ps_mm rotation shallow (3 allocs per kc at most).
            def s4_full(q16):
                ps = ps_mm.tile([128, QT_W], F32, tag="mm", name="ps_s4")
                o_sb = op_.tile([128, D], F16, tag="o", name="o_sb")
                for hp in range(2):
                    for dc2 in range(2):
                        nc.tensor.matmul(
                            ps[:, dc2 * 512 : (dc2 + 1) * 512],
                            ot_sb[hp][:, q16 * 128 : (q16 + 1) * 128],
                            wo_sb[hp][:, dc2 * 512 : (dc2 + 1) * 512],
                            start=(hp == 0),
                            stop=(hp == 1),
                        )
                nc.vector.tensor_copy(o_sb[:], ps[:])
                nc.sync.dma_start(out.ap()[q16 * 128 : (q16 + 1) * 128, :], o_sb[:])

            def s4_half(q16, hp, copy_eng=None):
                ps = ps_mm.tile([128, QT_W], F32, tag="mm", name="ps_s4h")
                o_sb = op_.tile([128, D], F16, tag="o", name="o_sb")
                for dc2 in range(2):
                    nc.tensor.matmul(
                        ps[:, dc2 * 512 : (dc2 + 1) * 512],
                        ot_sb[hp][:, q16 * 128 : (q16 + 1) * 128],
                        wo_sb[hp][:, dc2 * 512 : (dc2 + 1) * 512],
                        start=True,
                        stop=True,
                    )
                (copy_eng or nc.vector.tensor_copy)(o_sb[:], ps[:])
                if hp == 0:
                    nc.sync.dma_start(
                        out.ap()[q16 * 128 : (q16 + 1) * 128, :], o_sb[:]
                    )
                else:
                    nc.sync.dma_start(out2.ap()[q16 * 128 : (q16 + 1) * 128, :], o_sb[:])

            # ---- prologue stage-2 work: just enough for strand 0 kc0
            qk_unit(wk_sb, kt_sb, 0, 0)
            qk_unit(wq_sb, qt_sb, 0, 0)
            qk_unit(wq_sb, qt_sb, 0, 1)

            # ---- filler schedule: (strand_idx, kc) -> list of thunks.
            # PSUM-alloc parity: the per-kc scores tiles take 2 ps_mm allocs,
            # so filler units that allocate ps_mm must come in PAIRS per kc —
            # an odd count would make the next kc's scores alloc wait on a
            # filler's cast (PE->DVE->PE serialization) instead of on exp.
            fillers = {}

            def F(si, kc, fn):
                fillers.setdefault((si, kc), []).append(fn)

            def P(si, kc, fa, fb):
                F(si, kc, fa)
                F(si, kc, fb)

            # strand 0 (qt0,hp0): rest of K(hp0) [self, kc4+], Q(hp0) sc2-3
            # [strand 1 reads qt1 cols], all 16 V chunks [PV lag 2], and the
            # hp1-tile pad zeros [strand 2] (no ps_mm allocs)
            P(0, 0, lambda: qk_unit(wk_sb, kt_sb, 0, 1), lambda: v_unit(0))
            P(0, 1, lambda: v_unit(1), lambda: v_unit(2))
            P(0, 2, lambda: qk_unit(wk_sb, kt_sb, 0, 2), lambda: v_unit(3))
            P(0, 3, lambda: v_unit(4), lambda: v_unit(5))
            P(0, 4, lambda: qk_unit(wk_sb, kt_sb, 0, 3), lambda: v_unit(6))
            P(0, 5, lambda: v_unit(7), lambda: v_unit(8))
            P(0, 6, lambda: qk_unit(wq_sb, qt_sb, 0, 2), lambda: v_unit(9))
            P(0, 7, lambda: v_unit(10), lambda: v_unit(11))
            P(0, 8, lambda: qk_unit(wq_sb, qt_sb, 0, 3), lambda: v_unit(12))
            P(0, 9, lambda: v_unit(13), lambda: v_unit(14))
            P(0, 10, lambda: v_unit(15), lambda: pe_warm(1))
            F(0, 11, lambda: pad_zero(kt_sb[1], nc.vector.tensor_copy))
            F(0, 12, lambda: pad_zero(qt_sb[1], nc.vector.tensor_copy))
            # strand 1 (qt1,hp0): K(hp1) all sc + Q(hp1) sc2-3 [strand 2]
            P(1, 1, lambda: qk_unit(wk_sb, kt_sb, 1, 0),
              lambda: qk_unit(wk_sb, kt_sb, 1, 1))
            P(1, 5, lambda: qk_unit(wk_sb, kt_sb, 1, 2),
              lambda: qk_unit(wk_sb, kt_sb, 1, 3))
            P(1, 9, lambda: qk_unit(wq_sb, qt_sb, 1, 2),
              lambda: qk_unit(wq_sb, qt_sb, 1, 3))
            # strand 2 (qt1,hp1): Q(hp1) sc0-1 [strand 3], then the hp0 half
            # of the qt0 projection (strand 0 norm long done)
            P(2, 1, lambda: qk_unit(wq_sb, qt_sb, 1, 0),
              lambda: qk_unit(wq_sb, qt_sb, 1, 1))
            for u in range(4):
                P(2, 5 + 2 * u, lambda q16=2 * u: s4_half(q16, 0),
                  lambda q16=2 * u + 1: s4_half(q16, 0))
            # strand 3 (qt0,hp1): full qt1 projection; first pair waits until
            # kc4 so strand 2's normalize (their ot dep) is settled without
            # blocking the PE queue head
            for u in range(4):
                P(3, 4 + 2 * u, lambda q16=8 + 2 * u: s4_full(q16),
                  lambda q16=9 + 2 * u: s4_full(q16))

            # ---- attention strands: (qt, hp) ordered so only the qt0/hp1
            # projection half is left for the tail
            strands = [(0, 0), (1, 0), (1, 1), (0, 1)]
            PV_LAG = 2

            for si, (qt, hp) in enumerate(strands):
                accs = {}
                for hsel in range(2):
                    for j in range(2):
                        accs[hsel, j] = ps_acc.tile(
                            [128, 512], F32, tag="acc", name=f"acc{hsel}{j}"
                        )

                def pv(kc, es, accs=accs, hp=hp):
                    for hsel in range(2):
                        h = hp * 2 + hsel
                        lhsT = vp_sb[
                            :, (kc * HPC + h) * V_W : (kc * HPC + h) * V_W + V_W
                        ]
                        for j in range(2):
                            nc.tensor.matmul(
                                accs[hsel, j][0:V_W, :],
                                lhsT,
                                es[hsel][:, j * 512 : (j + 1) * 512],
                                start=(kc == 0),
                                stop=(kc == KC - 1),
                            )

                pending = []
                for kc in range(KC):
                    es = []
                    for hsel in range(2):
                        sc_ps = ps_mm.tile([128, QT_W], F32, tag="mm")
                        for j in range(2):
                            nc.tensor.matmul(
                                sc_ps[:, j * 512 : (j + 1) * 512],
                                kt_sb[hp][
                                    :, hsel * S + kc * 128 : hsel * S + (kc + 1) * 128
                                ],
                                qt_sb[hp][
                                    :,
                                    hsel * S
                                    + qt * QT_W
                                    + j * 512 : hsel * S
                                    + qt * QT_W
                                    + (j + 1) * 512,
                                ],
                                start=True,
                                stop=True,
                            )
                        e_sb = ep.tile([128, QT_W], F16, tag="e")
                        nc.scalar.activation(
                            e_sb[:], sc_ps[:], mybir.ActivationFunctionType.Exp
                        )
                        es.append(e_sb)
                    pending.append((kc, es))
                    if len(pending) > PV_LAG:
                        pv(*pending.pop(0))
                    for u in fillers.get((si, kc), ()):
                        u()
                for item in pending:
                    pv(*item)

                # keep the PE_HAM window busy across the normalize latency
                # (the tail after strand 3 has no real PE work to chase);
                # two calls keep the ps_mm alloc count even
                pe_warm(5 if si == 3 else 2)
                pe_warm(5 if si == 3 else 2)
                # normalize, software-pipelined across the 4 accumulators
                quads = [(hsel, j) for j in range(2) for hsel in range(2)]
                dens, rs, rbs = {}, {}, {}
                for hsel, j in quads:
                    den_sb = np_.tile([1, 512], F32, tag="den", name=f"den{hsel}{j}")
                    nc.vector.tensor_copy(den_sb[:], accs[hsel, j][DK : DK + 1, :])
                    dens[hsel, j] = den_sb
                for hsel, j in quads:
                    r_sb = np_.tile([1, 512], F32, tag="r", name=f"r{hsel}{j}")
                    nc.vector.reciprocal_approx_fast(r_sb[:], dens[hsel, j][:])
                    rs[hsel, j] = r_sb
                for hsel, j in quads:
                    rb_sb = np_.tile([64, 512], F32, tag="rb", name=f"rb{hsel}{j}")
                    nc.gpsimd.partition_broadcast(rb_sb[:], rs[hsel, j][:])
                    rbs[hsel, j] = rb_sb
                for hsel, j in quads:
                    q0 = qt * QT_W + j * 512
                    nc.vector.tensor_mul(
                        ot_sb[hp][hsel * 64 : hsel * 64 + 64, q0 : q0 + 512],
                        accs[hsel, j][0:DK, :],
                        rbs[hsel, j][:],
                    )

            # ---- tail: hp1 half of qt0 projection rows. PSUM->SBUF copies
            # run on ScalarE (idle once the exps are done) so the tail isn't
            # serialized behind the normalize work on DVE.
            for q16 in range(8):
                s4_half(q16, 1, copy_eng=nc.scalar.copy)

    nc.compile()
    return nc


def _shard_inputs(x, W_q, W_k, W_v, W_o):
    """Build the 8 per-core input maps (fp16, C-contiguous)."""

    def pack_w(w_rows):  # [256, D] weight rows -> [128, DC*256] lhsT tiles
        wt = w_rows.T.astype(np.float16)  # [D, 256]
        return np.ascontiguousarray(
            wt.reshape(DC, 128, 256).transpose(1, 0, 2).reshape(128, DC * 256)
        )

    in_maps = []
    for c in range(NCORES):
        b, g = divmod(c, HPC)
        rows = slice(g * HPC * DK, (g + 1) * HPC * DK)
        xt = x[b].T.astype(np.float16)  # [D, S]
        xs = np.ascontiguousarray(
            xt.reshape(DC, 128, SC, 512).transpose(2, 1, 0, 3).reshape(SC, 128, DC * 512)
        )
        in_maps.append(
            {
                "xs": xs,
                "wq": pack_w(W_q[rows] * 0.125),
                "wk": pack_w(W_k[rows]),
                "wv": pack_w(W_v[rows]),
                "wo": np.ascontiguousarray(
                    W_o[:, rows].T.astype(np.float16).reshape(2, 128, D)
                ),
            }
        )
    return in_maps


def _numpy_fallback(x, attention_mask, W_q, W_k, W_v, W_o):
    """Exact reference path (only used if the mask is not all ones)."""
    out = np.empty((B, S, D), np.float32)
    for b in range(B):
        q = (x[b] @ W_q.T).reshape(S, H, DK).transpose(1, 0, 2)
        k = (x[b] @ W_k.T).reshape(S, H, DK).transpose(1, 0, 2)
        v = (x[b] @ W_v.T).reshape(S, H, DK).transpose(1, 0, 2)
        scores = np.einsum("hqd,hkd->hqk", q, k)
        scores = np.where(attention_mask[b][None, None, :] == 0, -np.inf, scores)
        scores = scores / np.sqrt(DK)
        scores -= scores.max(axis=-1, keepdims=True)
        w = np.exp(scores)
        w /= w.sum(axis=-1, keepdims=True)
        o = np.einsum("hqk,hkd->hqd", w, v).transpose(1, 0, 2).reshape(S, D)
        out[b] = o @ W_o.T
    return out


def kernel(x, attention_mask, W_q, W_k, W_v, W_o, _trace=False):
    global _CACHED_NC
    x = np.asarray(x, dtype=np.float32)
    attention_mask = np.asarray(attention_mask)
    W_q = np.asarray(W_q, dtype=np.float32)
    W_k = np.asarray(W_k, dtype=np.float32)
    W_v = np.asarray(W_v, dtype=np.float32)
    W_o = np.asarray(W_o, dtype=np.float32)

    if not np.all(attention_mask == 1):
        return _numpy_fallback(x, attention_mask, W_q, W_k, W_v, W_o)

    if _CACHED_NC is None:
        _CACHED_NC = _build_nc()
    nc = _CACHED_NC

    in_maps = _shard_inputs(x, W_q, W_k, W_v, W_o)
    res = run_bass_kernel_spmd(
        nc, in_maps, core_ids=list(range(NCORES)), trace=_trace
    )

    out = np.empty((B, S, D), np.float32)
    for b in range(B):
        acc = np.zeros((S, D), np.float64)
        for g in range(HPC):
            r = res.results[b * HPC + g]
            acc += r["out"].astype(np.float64)
            acc[:QT_W] += r["out2"].astype(np.float64)
        out[b] = acc.astype(np.float32)
    if _trace:
        kernel.last_exec_time_ns = res.exec_time_ns
    return out


# revision 18
# speedup vs baseline: 1.0857x; 1.0254x over previous
"""Multi-head attention (B=2, S=2048, D=1024, H=16) on 8 Trainium2 NeuronCores.

Sharding: core c = (batch b = c//4) x (head-group g = c%4, 4 heads each).
Each core computes its 4 heads' attention plus the partial output
projection over its 256 W_o rows; the host sums the group partials.

All matmuls run in fp16 (end-to-end max rel err ~1e-3 vs the fp32
reference). PSUM accumulation is fp32.

v2 layout: every matmul runs in the PE's (128,128) tiling mode so the
kc loop never pays a tiling-mode-switch drain. The per-head K tiles are
stored zero-PADDED to 128 contraction rows (head 2g at partitions 0-63
with zeros below, head 2g+1 at partitions 64-127 with zeros above, in
disjoint column halves); the padded rows multiply garbage Q rows by
zero, so only the lhsT (K) side needs the memset. Scores stream time is
column-bound, so the padding costs nothing.

Emission plan: the PE is the bottleneck engine (~164us of matmul column
streams vs ~143us of ScalarE exp), so stage-2 (QKV) and stage-4 (W_o)
work is chopped into self-contained ~0.4-0.9us pieces and dropped into
the attention kc loop so the PE never idles. Strands run qt-major so
output-projection pieces unlock as early as possible; the last quarter
of the projection is computed per-hp (host adds the extra partial) to
shrink the no-exp tail.
"""

import sys

for _p in ("/opt/trn_rl_repo", "/root/.axon_site/_ro/trn_rl_repo"):
    if _p not in sys.path:
        sys.path.insert(0, _p)

import numpy as np

import concourse.mybir as mybir
import concourse.tile as tile
from concourse import bacc
from concourse.bass_utils import run_bass_kernel_spmd

F32 = mybir.dt.float32
F16 = mybir.dt.float16

B, S, D = 2, 2048, 1024
H, DK = 16, 64
HPC = 4          # heads per core
NCORES = 8
DC = 8           # number of 128-row chunks of D (contraction tiles)
SC = 4           # S chunks of 512 for the projections
QT_W = 1024      # q-tile width in stage 3
KC = S // 128    # 16 k-chunks
V_W = DK + 1     # 65: V columns per head incl. fused ones column

_CACHED_NC = None


def _build_nc():
    nc = bacc.Bacc("TRN2", target_bir_lowering=False, debug=False)

    xs = nc.dram_tensor("xs", [SC, 128, DC * 512], F16, kind="ExternalInput")
    wq = nc.dram_tensor("wq", [128, DC * 2 * 128], F16, kind="ExternalInput")
    wk = nc.dram_tensor("wk", [128, DC * 2 * 128], F16, kind="ExternalInput")
    wv = nc.dram_tensor("wv", [128, DC * HPC * DK], F16, kind="ExternalInput")
    wo = nc.dram_tensor("wo", [2, 128, D], F16, kind="ExternalInput")
    out = nc.dram_tensor("out", [S, D], F16, kind="ExternalOutput")
    # hp=1 partial of the qt=0 rows (host adds it onto out[:1024])
    out2 = nc.dram_tensor("out2", [QT_W, D], F16, kind="ExternalOutput")

    with tile.TileContext(nc) as tc:
        with (
            tc.tile_pool(name="persist", bufs=1) as pp,
            tc.tile_pool(name="ps_mm", bufs=2, space="PSUM") as ps_mm,
            tc.tile_pool(name="ps_acc", bufs=4, space="PSUM") as ps_acc,
            tc.tile_pool(name="exp_pool", bufs=8) as ep,
            tc.tile_pool(name="out_pool", bufs=4) as op_,
            tc.tile_pool(name="nrm_pool", bufs=5) as np_,
        ):
            # ---- ScalarE act-table preload: dummy exp before anything else
            warm_in = pp.tile([128, 1], F32, tag="warm_i")
            warm_out = pp.tile([128, 1], F16, tag="warm_o")
            nc.gpsimd.memset(warm_in[:], 0.0)
            nc.scalar.activation(
                warm_out[:], warm_in[:], mybir.ActivationFunctionType.Exp
            )

            # ---- input DMAs, ordered so the first strand's deps land first
            wk_sb = pp.tile([128, DC * 256], F16, tag="wk")
            nc.sync.dma_start(wk_sb[:], wk.ap())
            x_sb = [
                pp.tile([128, DC * 512], F16, tag=f"x{i}", name=f"x_sb{i}")
                for i in range(SC)
            ]
            nc.sync.dma_start(x_sb[0][:], xs.ap()[0])
            wq_sb = pp.tile([128, DC * 256], F16, tag="wq")
            nc.sync.dma_start(wq_sb[:], wq.ap())
            nc.sync.dma_start(x_sb[1][:], xs.ap()[1])
            wv_sb = pp.tile([128, DC * 256], F16, tag="wv")
            nc.sync.dma_start(wv_sb[:], wv.ap())
            nc.sync.dma_start(x_sb[2][:], xs.ap()[2])
            nc.sync.dma_start(x_sb[3][:], xs.ap()[3])
            wo_sb = [
                pp.tile([128, D], F16, tag=f"wo{i}", name=f"wo_sb{i}")
                for i in range(2)
            ]
            for i in range(2):
                nc.sync.dma_start(wo_sb[i][:], wo.ap()[i])

            # ---- per-head-pair Q/K tiles, 2 heads in disjoint column halves
            # head 2g at partitions 0-63 (cols 0:S), head 2g+1 at partitions
            # 64-127 (cols S:2S). kt needs zeros in the complement rows (it is
            # the matmul lhsT); qt complement rows are never read as nonzero
            # weights so they can stay garbage.
            qt_sb = [
                pp.tile([128, 2 * S], F16, tag=f"qt{i}", name=f"qt_sb{i}")
                for i in range(2)
            ]
            kt_sb = [
                pp.tile([128, 2 * S], F16, tag=f"kt{i}", name=f"kt_sb{i}")
                for i in range(2)
            ]
            vp_sb = pp.tile([128, KC * HPC * V_W], F16, tag="vp")
            ot_sb = [
                pp.tile([128, S], F16, tag=f"ot{i}", name=f"ot_sb{i}")
                for i in range(2)
            ]

            # dummy matmuls on kt garbage (result discarded): keep the PE_HAM
            # activity window busy (idle >=3.4us re-throttles the PE clock to
            # 1.2GHz). No data deps, so they issue from t~0.
            def pe_warm(n):
                ps = ps_mm.tile([128, 512], F32, tag="mm", name="ps_warm")
                for i in range(n):
                    nc.tensor.matmul(
                        ps[:],
                        kt_sb[0][:, 0:128],
                        kt_sb[0][:, 0:512],
                        start=True,
                        stop=True,
                    )

            # warm the PE during the input-DMA wait so the first real matmul
            # already runs at 2.4GHz
            pe_warm(6)
            pe_warm(6)

            # zero scratch -> kt/qt pad halves (bounce via f32: memset is
            # f32-only). kt pads make the padded contraction exact; qt pads
            # guard against Inf/NaN garbage turning 0*garbage into NaN.
            # hp0 tiles are needed by strand 0: their zeros go on ScalarE
            # (idle until the first exp) + DVE; hp1 tiles are zeroed by
            # strand-0 fillers.
            zero_sb = pp.tile([128, 1024], F32, tag="zero")
            nc.gpsimd.memset(zero_sb[:], 0.0)

            def pad_zero(t_sb, eng):
                for half in range(2):
                    eng(
                        t_sb[64:128, half * 1024 : (half + 1) * 1024],
                        zero_sb[64:128, :],
                    )
                for half in range(2):
                    eng(
                        t_sb[0:64, S + half * 1024 : S + (half + 1) * 1024],
                        zero_sb[0:64, :],
                    )

            pad_zero(kt_sb[0], nc.scalar.copy)
            pad_zero(qt_sb[0], nc.vector.tensor_copy)

            # ones columns of V'
            ones_sb = pp.tile([128, KC * HPC], F32, tag="ones")
            nc.gpsimd.memset(ones_sb[:], 1.0)
            ones_ap = vp_sb[:].rearrange("p (c g) -> p c g", g=V_W)[:, :, DK : DK + 1]
            nc.vector.tensor_copy(ones_ap, ones_sb[:].unsqueeze(-1))

            # ---- stage-2 / stage-4 work units. Each unit is self-contained
            # (allocates and releases its PSUM slot within the unit) so a
            # unit can sit anywhere in the PE queue without deadlocking the
            # ps_mm rotation.
            def qk_unit(w_sb, t_sb, hp, sc):
                """Q or K projection for one head pair over 512 queries."""
                ps = ps_mm.tile([128, 512], F32, tag="mm", name="ps_qk")
                for d in range(DC):
                    nc.tensor.matmul(
                        ps[:],
                        w_sb[:, d * 256 + hp * 128 : d * 256 + hp * 128 + 128],
                        x_sb[sc][:, d * 512 : (d + 1) * 512],
                        start=(d == 0),
                        stop=(d == DC - 1),
                    )
                # rows 0-63 = head 2g -> cols [sc*512 ..], rows 64-127 =
                # head 2g+1 -> same cols offset by S
                nc.vector.tensor_copy(
                    t_sb[hp][0:64, sc * 512 : (sc + 1) * 512], ps[0:64, :]
                )
                nc.vector.tensor_copy(
                    t_sb[hp][64:128, S + sc * 512 : S + (sc + 1) * 512],
                    ps[64:128, :],
                )

            def v_unit(kc):
                sc, i = divmod(kc, 4)
                ps = ps_mm.tile([128, 512], F32, tag="mm", name="ps_v")
                for d in range(DC):
                    nc.tensor.matmul(
                        ps[:, 0 : HPC * DK],
                        x_sb[sc][:, d * 512 + i * 128 : d * 512 + i * 128 + 128],
                        wv_sb[:, d * 256 : (d + 1) * 256],
                        start=(d == 0),
                        stop=(d == DC - 1),
                    )
                dst = vp_sb[:, kc * V_W * HPC : (kc + 1) * V_W * HPC]
                dst = dst.rearrange("p (g c) -> p g c", c=V_W)[:, :, 0:DK]
                src = ps[:, 0 : HPC * DK].rearrange("p (g c) -> p g c", c=DK)
                nc.vector.tensor_copy(dst, src)

            # stage 4: full unit (hp-summed on device) for qt1 rows; per-hp
            # half for qt0 rows (host adds the hp=1 partial from out2).
            # One [128,1024] PSUM alloc + one cast + one DMA per unit keeps
            # the ps_mm rotation shallow (3 allocs per kc at most).
            def s4_full(q16):
                ps = ps_mm.tile([128, QT_W], F32, tag="mm", name="ps_s4")
                o_sb = op_.tile([128, D], F16, tag="o", name="o_sb")
                for hp in range(2):
                    for dc2 in range(2):
                        nc.tensor.matmul(
                            ps[:, dc2 * 512 : (dc2 + 1) * 512],
                            ot_sb[hp][:, q16 * 128 : (q16 + 1) * 128],
                            wo_sb[hp][:, dc2 * 512 : (dc2 + 1) * 512],
                            start=(hp == 0),
                            stop=(hp == 1),
                        )
                nc.vector.tensor_copy(o_sb[:], ps[:])
                nc.sync.dma_start(out.ap()[q16 * 128 : (q16 + 1) * 128, :], o_sb[:])

            def s4_half(q16, hp, copy_eng=None):
                ps = ps_mm.tile([128, QT_W], F32, tag="mm", name="ps_s4h")
                o_sb = op_.tile([128, D], F16, tag="o", name="o_sb")
                for dc2 in range(2):
                    nc.tensor.matmul(
                        ps[:, dc2 * 512 : (dc2 + 1) * 512],
                        ot_sb[hp][:, q16 * 128 : (q16 + 1) * 128],
                        wo_sb[hp][:, dc2 * 512 : (dc2 + 1) * 512],
                        start=True,
                        stop=True,
                    )
                (copy_eng or nc.vector.tensor_copy)(o_sb[:], ps[:])
                if hp == 0:
                    nc.sync.dma_start(
                        out.ap()[q16 * 128 : (q16 + 1) * 128, :], o_sb[:]
                    )
                else:
                    nc.sync.dma_start(out2.ap()[q16 * 128 : (q16 + 1) * 128, :], o_sb[:])

            # ---- prologue stage-2 work: just enough for strand 0 kc0
            qk_unit(wk_sb, kt_sb, 0, 0)
            qk_unit(wq_sb, qt_sb, 0, 0)
            qk_unit(wq_sb, qt_sb, 0, 1)

            # ---- filler schedule: (strand_idx, kc) -> list of thunks.
            # PSUM-alloc parity: the per-kc scores tiles take 2 ps_mm allocs,
            # so filler units that allocate ps_mm must come in PAIRS per kc —
            # an odd count would make the next kc's scores alloc wait on a
            # filler's cast (PE->DVE->PE serialization) instead of on exp.
            fillers = {}

            def F(si, kc, fn):
                fillers.setdefault((si, kc), []).append(fn)

            def P(si, kc, fa, fb):
                F(si, kc, fa)
                F(si, kc, fb)

            # strand 0 (qt0,hp0): rest of K(hp0) [self, kc4+], Q(hp0) sc2-3
            # [strand 1 reads qt1 cols], all 16 V chunks [PV lag 2], and the
            # hp1-tile pad zeros [strand 2] (no ps_mm allocs)
            P(0, 0, lambda: qk_unit(wk_sb, kt_sb, 0, 1), lambda: v_unit(0))
            P(0, 1, lambda: v_unit(1), lambda: v_unit(2))
            P(0, 2, lambda: qk_unit(wk_sb, kt_sb, 0, 2), lambda: v_unit(3))
            P(0, 3, lambda: v_unit(4), lambda: v_unit(5))
            P(0, 4, lambda: qk_unit(wk_sb, kt_sb, 0, 3), lambda: v_unit(6))
            P(0, 5, lambda: v_unit(7), lambda: v_unit(8))
            P(0, 6, lambda: qk_unit(wq_sb, qt_sb, 0, 2), lambda: v_unit(9))
            P(0, 7, lambda: v_unit(10), lambda: v_unit(11))
            P(0, 8, lambda: qk_unit(wq_sb, qt_sb, 0, 3), lambda: v_unit(12))
            P(0, 9, lambda: v_unit(13), lambda: v_unit(14))
            P(0, 10, lambda: v_unit(15), lambda: pe_warm(1))
            F(0, 11, lambda: pad_zero(kt_sb[1], nc.vector.tensor_copy))
            F(0, 12, lambda: pad_zero(qt_sb[1], nc.vector.tensor_copy))
            # strand 1 (qt1,hp0): K(hp1) all sc + Q(hp1) sc2-3 [strand 2]
            P(1, 1, lambda: qk_unit(wk_sb, kt_sb, 1, 0),
              lambda: qk_unit(wk_sb, kt_sb, 1, 1))
            P(1, 5, lambda: qk_unit(wk_sb, kt_sb, 1, 2),
              lambda: qk_unit(wk_sb, kt_sb, 1, 3))
            P(1, 9, lambda: qk_unit(wq_sb, qt_sb, 1, 2),
              lambda: qk_unit(wq_sb, qt_sb, 1, 3))
            # strand 2 (qt1,hp1): Q(hp1) sc0-1 [strand 3], then the hp0 half
            # of the qt0 projection (strand 0 norm long done)
            P(2, 1, lambda: qk_unit(wq_sb, qt_sb, 1, 0),
              lambda: qk_unit(wq_sb, qt_sb, 1, 1))
            for u in range(4):
                P(2, 5 + 2 * u, lambda q16=2 * u: s4_half(q16, 0),
                  lambda q16=2 * u + 1: s4_half(q16, 0))
            # strand 3 (qt0,hp1): full qt1 projection; first pair waits until
            # kc4 so strand 2's normalize (their ot dep) is settled without
            # blocking the PE queue head
            for u in range(4):
                P(3, 4 + 2 * u, lambda q16=8 + 2 * u: s4_full(q16),
                  lambda q16=9 + 2 * u: s4_full(q16))

            # ---- attention strands: (qt, hp) ordered so only the qt0/hp1
            # projection half is left for the tail
            strands = [(0, 0), (1, 0), (1, 1), (0, 1)]
            PV_LAG = 2

            for si, (qt, hp) in enumerate(strands):
                accs = {}
                for hsel in range(2):
                    for j in range(2):
                        accs[hsel, j] = ps_acc.tile(
                            [128, 512], F32, tag="acc", name=f"acc{hsel}{j}"
                        )

                def pv(kc, es, accs=accs, hp=hp):
                    for hsel in range(2):
                        h = hp * 2 + hsel
                        lhsT = vp_sb[
                            :, (kc * HPC + h) * V_W : (kc * HPC + h) * V_W + V_W
                        ]
                        for j in range(2):
                            nc.tensor.matmul(
                                accs[hsel, j][0:V_W, :],
                                lhsT,
                                es[hsel][:, j * 512 : (j + 1) * 512],
                                start=(kc == 0),
                                stop=(kc == KC - 1),
                            )

                pending = []
                for kc in range(KC):
                    es = []
                    for hsel in range(2):
                        sc_ps = ps_mm.tile([128, QT_W], F32, tag="mm")
                        for j in range(2):
                            nc.tensor.matmul(
                                sc_ps[:, j * 512 : (j + 1) * 512],
                                kt_sb[hp][
                                    :, hsel * S + kc * 128 : hsel * S + (kc + 1) * 128
                                ],
                                qt_sb[hp][
                                    :,
                                    hsel * S
                                    + qt * QT_W
                                    + j * 512 : hsel * S
                                    + qt * QT_W
                                    + (j + 1) * 512,
                                ],
                                start=True,
                                stop=True,
                            )
                        e_sb = ep.tile([128, QT_W], F16, tag="e")
                        nc.scalar.activation(
                            e_sb[:], sc_ps[:], mybir.ActivationFunctionType.Exp
                        )
                        es.append(e_sb)
                    pending.append((kc, es))
                    if len(pending) > PV_LAG:
                        pv(*pending.pop(0))
                    for u in fillers.get((si, kc), ()):
                        u()
                for item in pending:
                    pv(*item)

                # keep the PE_HAM window busy across the normalize latency
                # (the tail after strand 3 has no real PE work to chase);
                # two calls keep the ps_mm alloc count even
                pe_warm(8 if si == 3 else 2)
                pe_warm(8 if si == 3 else 2)
                # normalize, software-pipelined across the 4 accumulators
                quads = [(hsel, j) for j in range(2) for hsel in range(2)]
                dens, rs, rbs = {}, {}, {}
                for hsel, j in quads:
                    den_sb = np_.tile([1, 512], F32, tag="den", name=f"den{hsel}{j}")
                    nc.vector.tensor_copy(den_sb[:], accs[hsel, j][DK : DK + 1, :])
                    dens[hsel, j] = den_sb
                for hsel, j in quads:
                    r_sb = np_.tile([1, 512], F32, tag="r", name=f"r{hsel}{j}")
                    nc.vector.reciprocal_approx_fast(r_sb[:], dens[hsel, j][:])
                    rs[hsel, j] = r_sb
                for hsel, j in quads:
                    rb_sb = np_.tile([64, 512], F32, tag="rb", name=f"rb{hsel}{j}")
                    nc.gpsimd.partition_broadcast(rb_sb[:], rs[hsel, j][:])
                    rbs[hsel, j] = rb_sb
                for hsel, j in quads:
                    q0 = qt * QT_W + j * 512
                    nc.vector.tensor_mul(
                        ot_sb[hp][hsel * 64 : hsel * 64 + 64, q0 : q0 + 512],
                        accs[hsel, j][0:DK, :],
                        rbs[hsel, j][:],
                    )

            # ---- tail: hp1 half of qt0 projection rows. PSUM->SBUF copies
            # run on ScalarE (idle once the exps are done) so the tail isn't
            # serialized behind the normalize work on DVE.
            for q16 in range(8):
                s4_half(q16, 1, copy_eng=nc.scalar.copy)

    nc.compile()
    return nc


def _shard_inputs(x, W_q, W_k, W_v, W_o):
    """Build the 8 per-core input maps (fp16, C-contiguous)."""

    def pack_w(w_rows):  # [256, D] weight rows -> [128, DC*256] lhsT tiles
        wt = w_rows.T.astype(np.float16)  # [D, 256]
        return np.ascontiguousarray(
            wt.reshape(DC, 128, 256).transpose(1, 0, 2).reshape(128, DC * 256)
        )

    in_maps = []
    for c in range(NCORES):
        b, g = divmod(c, HPC)
        rows = slice(g * HPC * DK, (g + 1) * HPC * DK)
        xt = x[b].T.astype(np.float16)  # [D, S]
        xs = np.ascontiguousarray(
            xt.reshape(DC, 128, SC, 512).transpose(2, 1, 0, 3).reshape(SC, 128, DC * 512)
        )
        in_maps.append(
            {
                "xs": xs,
                "wq": pack_w(W_q[rows] * 0.125),
                "wk": pack_w(W_k[rows]),
                "wv": pack_w(W_v[rows]),
                "wo": np.ascontiguousarray(
                    W_o[:, rows].T.astype(np.float16).reshape(2, 128, D)
                ),
            }
        )
    return in_maps


def _numpy_fallback(x, attention_mask, W_q, W_k, W_v, W_o):
    """Exact reference path (only used if the mask is not all ones)."""
    out = np.empty((B, S, D), np.float32)
    for b in range(B):
        q = (x[b] @ W_q.T).reshape(S, H, DK).transpose(1, 0, 2)
        k = (x[b] @ W_k.T).reshape(S, H, DK).transpose(1, 0, 2)
        v = (x[b] @ W_v.T).reshape(S, H, DK).transpose(1, 0, 2)
        scores = np.einsum("hqd,hkd->hqk", q, k)
        scores = np.where(attention_mask[b][None, None, :] == 0, -np.inf, scores)
        scores = scores / np.sqrt(DK)
        scores -= scores.max(axis=-1, keepdims=True)
        w = np.exp(scores)
        w /= w.sum(axis=-1, keepdims=True)
        o = np.einsum("hqk,hkd->hqd", w, v).transpose(1, 0, 2).reshape(S, D)
        out[b] = o @ W_o.T
    return out


def kernel(x, attention_mask, W_q, W_k, W_v, W_o, _trace=False):
    global _CACHED_NC
    x = np.asarray(x, dtype=np.float32)
    attention_mask = np.asarray(attention_mask)
    W_q = np.asarray(W_q, dtype=np.float32)
    W_k = np.asarray(W_k, dtype=np.float32)
    W_v = np.asarray(W_v, dtype=np.float32)
    W_o = np.asarray(W_o, dtype=np.float32)

    if not np.all(attention_mask == 1):
        return _numpy_fallback(x, attention_mask, W_q, W_k, W_v, W_o)

    if _CACHED_NC is None:
        _CACHED_NC = _build_nc()
    nc = _CACHED_NC

    in_maps = _shard_inputs(x, W_q, W_k, W_v, W_o)
    res = run_bass_kernel_spmd(
        nc, in_maps, core_ids=list(range(NCORES)), trace=_trace
    )

    out = np.empty((B, S, D), np.float32)
    for b in range(B):
        acc = np.zeros((S, D), np.float64)
        for g in range(HPC):
            r = res.results[b * HPC + g]
            acc += r["out"].astype(np.float64)
            acc[:QT_W] += r["out2"].astype(np.float64)
        out[b] = acc.astype(np.float32)
    if _trace:
        kernel.last_exec_time_ns = res.exec_time_ns
    return out
